# revision 17
# baseline (speedup 1.0000x reference)
"""Trainium2 Bass kernel for nn_DotProductScorer.

Computes, for ragged candidate tokens split into B segments by `starts`:
    q  = Linear(state_vec) = state_vec @ Wq.T + bq   [B, d_token]
    kq = q @ Wk.T                                    [B, d_token]
    logits[i] = dot(cand_tokens[i], kq[seg(i)])      for each token i
with tokens outside [starts[0], starts[-1]) zeroed.

Sharding: cand_tokens is sharded along the token axis K across 8 NeuronCores
(pure data parallel, per the sharding hint); the small Wq/bq/Wk params and the
per-core slice of state_vec ride along, and each core computes its kq rows
on-device with two small PE matmuls.

Fast path (uniform starts, SEG=512 — what reference.setup_inputs produces):
tokens are laid out segment-major (partition p of group g holds segment
g*128+p), and the 128 MiB cand shard is streamed through a 3-stage pipeline:

  1. Chunks stream through THREE DGE paths in rotation (see MIX below):
     most via SWDGE cast-DMA, every 3rd via the sync/scalar HWDGE rings as
     f32 + DVE cast.  A single SWDGE queue alone sustains only ~333-353
     GB/s; rotating descriptor generation across all three paths reaches
     ~350-365 GB/s (HBM-per-NC share is ~358-366).
     SWDGE (gpsimd) dma_start casts each f32 chunk to bf16 in flight.
     Reading f32 from HBM but writing half-width bf16 into SBUF lets the
     stream run at ~340 GB/s of HBM reads (measured) — faster than the
     same stream landing f32 (~320 GB/s), because the SBUF write side and
     descriptor work are halved.  The HBM read traffic itself (128 MiB) is
     the roofline; everything else is overlapped under it.
  2. One DVE tensor_tensor multiply per chunk against the segment's kq row
     (bf16, broadcast along the free axis with a stride-0 AP, in place).
     bf16 tensor_tensor runs the DVE in 2x_1P mode (245.8 G elem/s) — f32
     runs 1x — so the multiply costs ~137 us/core instead of ~274 us.
  3. Per-token reduction as an in-place pairwise add-tree over d (7 bf16
     tensor_tensor levels, the last fused into the L write).  The tree also
     runs 2x_1P (~9 us/chunk) whereas a flat tensor_reduce only has a 1x
     uop (~17 us/chunk), so the whole reduction fits on the DVE (~76% busy
     under the stream) and the ACT engine is not needed at all.
     (Alternatives kept selectable via `compute=`: 'split' = tensor_reduce
     on NR columns + ACT accumulate on the rest — the previous scheme —
     and 'treesplit' = one tree level then the split at d=64.  All three
     measured within ~2 us of each other; 'tree' was fastest and is the
     simplest dependency structure.)

Numerics: products are bf16 (inputs rounded to bf16, ~0.4% per element);
the add-tree rounds each of the 7 levels to bf16.  Measured end-to-end
relative error vs the f32 reference is 5.5e-3 (hardware, matching the
numpy simulation exactly), against the 2e-2 gate.

Perf model (HW-measured, interleaved differential benches): the pure cast
stream sustains ~353 GB/s/core (379.7 us for the 134.2 MB shard; the
HBM-per-NC share is ~358-366 GB/s), and the reduction stage adds a ~6 us
coupling tax that is insensitive to which engine does the work — 'tree',
'split' and 'treesplit' all land within noise of each other.  The kernel
is at the practical HBM roofline; the remaining ~2% gap between the
with-compute stream and the pure stream did not yield to engine-load
rebalancing (DVE 76% + ACT idle vs DVE 86% + ACT 70% time-identical).

General path (any sorted `starts`): host derives per-token segment ids and
expands kq to a per-token table E = kq[seg]; each core streams cand and E
shards through an f32 multiply + split reduction (exact, ~2x slower).
"""

import numpy as np

import concourse.bass as bass
import concourse.tile as tile
from concourse import bacc, mybir
from concourse.bass_utils import run_bass_kernel_spmd

B = 4096
SEG = 512
K = B * SEG
D_STATE = 256
D_TOKEN = 128
NCORES = 8
SEGS_PER_CORE = B // NCORES           # 512
TOK_PER_CORE = K // NCORES            # 262144

F32 = mybir.dt.float32
BF16 = mybir.dt.bfloat16
AF = mybir.ActivationFunctionType
ALU = mybir.AluOpType
AX = mybir.AxisListType

# fast-path tuning (selected by differential HW benchmarks)
OCH = 64       # token columns per chunk; chunk = [128, OCH, 128]
NR = 87        # 'split' mode: columns reduced by DVE; rest go to ACT
BUFS = 3       # bf16 chunk ring depth
FBUFS = 3      # f32 chunk ring depth (mix mode)
CBUFS = 2      # cast ring depth (mix mode)
COMPUTE = "tree"  # 'split' | 'tree' | 'treesplit' (see build_fast)
MIX = "22"     # '' = all chunks SWDGE bf16-cast; '22' = chunks i%4==1 / 3
               # stream f32 via the sync / scalar HWDGE rings instead and
               # are DVE-cast to bf16 on arrival.  Rotating the descriptor
               # work across all three DGE paths relieves the single SWDGE
               # queue: HW-measured 383.8 us vs 404.8 us all-SWDGE in the
               # same interleaved session (-5.2%).


F32RED = "tree"   # 'tree' | 'act': reduction scheme for f32-origin chunks
OUT_ENG = "sync"  # engine issuing the per-group L out-DMA
F32CAST = "act"   # 'dve' (tensor_copy 2x_2P) | 'act' (whole-chunk Copy)


def build_fast(och=OCH, nr=NR, bufs=BUFS, n_rep=1,
               segs_per_core=SEGS_PER_CORE, seg=SEG, compute=COMPUTE,
               mix=MIX, fbufs=FBUFS, cbufs=CBUFS, f32red=F32RED,
               out_eng=OUT_ENG, f32cast=F32CAST):
    """Uniform-starts program. Per core:
      inputs : svT [256, S] (state rows for this core's S segments, transposed)
               WqT [256,128], WkT [128,128], bq [128,1], cand [S*seg, 128]
      output : out [S*seg] f32
    Token layout: group g (128 segments), partition p = segment g*128+p,
    free index o in [0, seg) -> local token (g*128+p)*seg + o.
    n_rep > 1 re-runs the main stream over the same data (bench only)."""
    groups = segs_per_core // 128
    assert segs_per_core % 128 == 0 and seg % och == 0
    nchunk = seg // och
    tok = segs_per_core * seg

    nc = bacc.Bacc("TRN2", target_bir_lowering=False, debug=False,
                   num_devices=NCORES)
    svT = nc.dram_tensor("svT", [D_STATE, segs_per_core], F32,
                         kind="ExternalInput").ap()
    WqT = nc.dram_tensor("WqT", [D_STATE, D_TOKEN], F32,
                         kind="ExternalInput").ap()
    WkT = nc.dram_tensor("WkT", [D_TOKEN, D_TOKEN], F32,
                         kind="ExternalInput").ap()
    bqv = nc.dram_tensor("bq", [D_TOKEN, 1], F32, kind="ExternalInput").ap()
    cand = nc.dram_tensor("cand", [tok, D_TOKEN], F32,
                          kind="ExternalInput").ap()
    # logits land bf16 (host upcasts): halves the out-DMA.  Safe: DVE
    # tensor_reduce and the ACT accumulator both accumulate internally in
    # f32 and round once on the final write (HW-verified, ~1.3e-3).
    out = nc.dram_tensor("out", [tok], BF16, kind="ExternalOutput").ap()

    cand_r = cand.rearrange("(g p o) d -> g p o d", g=groups, p=128, o=seg)
    out_r = out.rearrange("(g p o) -> g p o", g=groups, p=128, o=seg)

    with tile.TileContext(nc) as tc:
        with (
            tc.tile_pool(name="const", bufs=1) as constp,
            tc.tile_pool(name="psum", bufs=2, space="PSUM") as psump,
            tc.tile_pool(name="chunk", bufs=bufs) as chunkp,
            tc.tile_pool(name="chunkf", bufs=fbufs) as chunkfp,
            tc.tile_pool(name="cast", bufs=cbufs) as castp,
            tc.tile_pool(name="lout", bufs=2) as loutp,
        ):
            # ---- prologue: kq = (sv @ Wq.T + bq) @ Wk.T, segment-major ----
            svT_t = constp.tile([128, 2, segs_per_core], F32)
            nc.sync.dma_start(svT_t[:, 0, :], svT[0:128, :])
            nc.sync.dma_start(svT_t[:, 1, :], svT[128:256, :])
            WqT_t = constp.tile([128, 2, D_TOKEN], F32)
            nc.sync.dma_start(WqT_t[:, 0, :], WqT[0:128, :])
            nc.sync.dma_start(WqT_t[:, 1, :], WqT[128:256, :])
            WkT_t = constp.tile([128, D_TOKEN], F32)
            nc.sync.dma_start(WkT_t[:], WkT[:])
            bq_t = constp.tile([128, 1], F32)
            nc.sync.dma_start(bq_t[:], bqv[:])

            # qT[d_tok, s] = sum_ds Wq[d_tok, ds] * sv[s, ds]
            qT_sb = constp.tile([128, segs_per_core], F32)
            for h in range(0, segs_per_core, 512):
                w = min(512, segs_per_core - h)
                qT_ps = psump.tile([128, 512], F32, tag="qT_ps")
                nc.tensor.matmul(qT_ps[:, :w], WqT_t[:, 0, :],
                                 svT_t[:, 0, h:h + w], start=True, stop=False)
                nc.tensor.matmul(qT_ps[:, :w], WqT_t[:, 1, :],
                                 svT_t[:, 1, h:h + w], start=False, stop=True)
                # + bq (per-partition bias) while copying PSUM -> SBUF
                nc.scalar.activation(qT_sb[:, h:h + w], qT_ps[:, :w],
                                     AF.Identity, bias=bq_t[:], scale=1.0)

            # kq[s, d2] = sum_d1 qT[d1, s] * WkT[d1, d2]; partition = segment.
            # Cast to bf16 on the PSUM->SBUF copy to match the cand stream.
            kq_sb = constp.tile([128, groups, D_TOKEN], BF16)
            for g in range(groups):
                kq_ps = psump.tile([128, D_TOKEN], F32, tag="kq_ps")
                nc.tensor.matmul(kq_ps[:], qT_sb[:, g * 128:(g + 1) * 128],
                                 WkT_t[:], start=True, stop=True)
                nc.scalar.copy(kq_sb[:, g, :], kq_ps[:])

            # ---- main: cast-stream + multiply + reduction ----
            # Reduction schemes (DVE tensor_reduce runs at 1x = 122.9 G
            # elem/s regardless of dtype; bf16 tensor_tensor runs 2x_1P =
            # 245.8 G elem/s; ACT accum ops cost (N+352)/1.2 ns each):
            #   split     : tensor_reduce on nr cols + ACT accum on the rest
            #   tree      : pairwise in-place add-tree over d at 2x rate,
            #               no ACT at all
            #   treesplit : one tree level (d 128->64), then split reduce of
            #               the halved columns between DVE and ACT
            ci = 0
            for _rep in range(n_rep):
                for g in range(groups):
                    L = loutp.tile([128, seg], BF16)
                    kq_b = kq_sb[:, g, :].unsqueeze(1).broadcast_to(
                        [128, och, D_TOKEN])
                    for kk in range(nchunk):
                        src = cand_r[g, :, kk * och:(kk + 1) * och, :]
                        feng = None
                        if mix == "62":
                            if ci % 6 == 2:
                                feng = nc.sync
                            elif ci % 6 == 5:
                                feng = nc.scalar
                        elif mix == "22":
                            if ci % 4 == 1:
                                feng = nc.sync
                            elif ci % 4 == 3:
                                feng = nc.scalar
                        elif mix == "3":
                            if ci % 3 == 1:
                                feng = nc.sync
                            elif ci % 3 == 2:
                                feng = nc.scalar
                        ci += 1
                        if feng is not None:
                            chf = chunkfp.tile([128, och, D_TOKEN], F32)
                            feng.dma_start(chf[:], src)
                            ch = castp.tile([128, och, D_TOKEN], BF16)
                            with nc.allow_low_precision(
                                    "bf16 cast, same rounding as the "
                                    "SWDGE in-flight cast"):
                                if f32cast == "act":
                                    nc.scalar.activation(
                                        ch[:], chf[:], AF.Copy,
                                        bias=0.0, scale=1.0)
                                else:
                                    nc.vector.tensor_copy(out=ch[:],
                                                          in_=chf[:])
                        else:
                            ch = chunkp.tile([128, och, D_TOKEN], BF16)
                            nc.gpsimd.dma_start(ch[:], src)
                        nc.vector.tensor_tensor(out=ch[:], in0=ch[:],
                                                in1=kq_b, op=ALU.mult)
                        ob = kk * och
                        if (compute == "tree" and feng is not None
                                and f32red == "act"):
                            # f32-origin chunks can reduce on the otherwise
                            # idle ACT engine (f32-internal accumulate)
                            for j in range(och):
                                with nc.allow_low_precision("f32-internal"):
                                    nc.scalar.activation(
                                        ch[:, j, :], ch[:, j, :], AF.Copy,
                                        bias=0.0, scale=1.0,
                                        accum_out=L[:, ob + j:ob + j + 1])
                            continue

                        def lp():
                            # fresh context each use (one-shot generator)
                            return nc.allow_low_precision(
                                "bf16 partials; verified 5.5e-3 vs 2e-2 gate")

                        if compute == "tree":
                            w = D_TOKEN
                            while w > 2:
                                h = w // 2
                                with lp():
                                    nc.vector.tensor_tensor(
                                        out=ch[:, :, 0:h],
                                        in0=ch[:, :, 0:h],
                                        in1=ch[:, :, h:w], op=ALU.add)
                                w = h
                            with lp():
                                nc.vector.tensor_tensor(
                                    out=L[:, ob:ob + och], in0=ch[:, :, 0],
                                    in1=ch[:, :, 1], op=ALU.add)
                            continue
                        red_d = D_TOKEN
                        if compute == "treesplit":
                            with lp():
                                nc.vector.tensor_tensor(
                                    out=ch[:, :, 0:64], in0=ch[:, :, 0:64],
                                    in1=ch[:, :, 64:128], op=ALU.add)
                            red_d = 64
                        if nr > 0:
                            with lp():
                                nc.vector.tensor_reduce(
                                    out=L[:, ob:ob + nr],
                                    in_=ch[:, 0:nr, 0:red_d],
                                    axis=AX.X, op=ALU.add)
                        for j in range(nr, och):
                            # pass-through out written in place so consecutive
                            # ACT ops don't WAW-serialize on a shared scratch
                            with nc.allow_low_precision("f32-internal"):
                                nc.scalar.activation(
                                    ch[:, j, 0:red_d], ch[:, j, 0:red_d],
                                    AF.Copy, bias=0.0, scale=1.0,
                                    accum_out=L[:, ob + j:ob + j + 1])
                    {"sync": nc.sync, "scalar": nc.scalar,
                     "gpsimd": nc.gpsimd}[out_eng].dma_start(
                        out_r[g, :, :], L[:])

    nc.compile()
    return nc


def build_general(tok_per_core=TOK_PER_CORE, och=64, nr=42, chunk_bufs=3):
    """Any-starts program. Per core:
      inputs : cand [T, 128], E [T, 128] (host-gathered kq[seg] rows,
               zeroed outside the valid range)
      output : out [T] f32
    Token layout: partition p handles tokens p*(T/128) .. (p+1)*(T/128).
    """
    assert tok_per_core % (128 * och) == 0
    a_len = tok_per_core // 128
    nchunk = a_len // och

    nc = bacc.Bacc("TRN2", target_bir_lowering=False, debug=False,
                   num_devices=NCORES)
    cand = nc.dram_tensor("cand", [tok_per_core, D_TOKEN], F32,
                          kind="ExternalInput").ap()
    ev = nc.dram_tensor("E", [tok_per_core, D_TOKEN], F32,
                        kind="ExternalInput").ap()
    out = nc.dram_tensor("out", [tok_per_core], F32,
                         kind="ExternalOutput").ap()

    cand_r = cand.rearrange("(p a) d -> p a d", p=128, a=a_len)
    e_r = ev.rearrange("(p a) d -> p a d", p=128, a=a_len)
    out_r = out.rearrange("(p a) -> p a", p=128, a=a_len)

    with tile.TileContext(nc) as tc:
        with (
            tc.tile_pool(name="chunk", bufs=chunk_bufs) as chunkp,
            tc.tile_pool(name="echunk", bufs=chunk_bufs) as echunkp,
            tc.tile_pool(name="lout", bufs=1) as loutp,
        ):
            L = loutp.tile([128, a_len], F32)
            for kk in range(nchunk):
                ch = chunkp.tile([128, och, D_TOKEN], F32)
                nc.sync.dma_start(ch[:], cand_r[:, kk * och:(kk + 1) * och, :])
                eh = echunkp.tile([128, och, D_TOKEN], F32)
                nc.sync.dma_start(eh[:], e_r[:, kk * och:(kk + 1) * och, :])
                nc.vector.tensor_tensor(out=ch[:], in0=ch[:], in1=eh[:],
                                        op=ALU.mult)
                if nr > 0:
                    nc.vector.tensor_reduce(out=L[:, kk * och:kk * och + nr],
                                            in_=ch[:, 0:nr, :], axis=AX.X,
                                            op=ALU.add)
                for j in range(nr, och):
                    nc.scalar.activation(ch[:, j, :], ch[:, j, :], AF.Copy,
                                         bias=0.0, scale=1.0,
                                         accum_out=L[:, kk * och + j:
                                                     kk * och + j + 1])
            nc.sync.dma_start(out_r[:, :], L[:])

    nc.compile()
    return nc


_PROG_CACHE = {}


def _get_prog(kind):
    if kind not in _PROG_CACHE:
        _PROG_CACHE[kind] = build_fast() if kind == "fast" else build_general()
    return _PROG_CACHE[kind]


def _is_uniform(starts):
    if starts.shape != (B + 1,):
        return False
    return bool(np.array_equal(starts.astype(np.int64),
                               np.arange(B + 1, dtype=np.int64) * SEG))


def fast_in_maps(state_vec, cand_tokens, Wq, bq, Wk):
    WqT = np.ascontiguousarray(Wq.T)                 # [256, 128]
    WkT = np.ascontiguousarray(Wk.T)                 # [128, 128]
    bq2 = np.ascontiguousarray(bq.reshape(D_TOKEN, 1))
    in_maps = []
    for c in range(NCORES):
        svT_c = np.ascontiguousarray(
            state_vec[c * SEGS_PER_CORE:(c + 1) * SEGS_PER_CORE].T)
        cand_c = cand_tokens[c * TOK_PER_CORE:(c + 1) * TOK_PER_CORE]
        in_maps.append({"svT": svT_c, "WqT": WqT, "WkT": WkT,
                        "bq": bq2, "cand": cand_c})
    return in_maps


def kernel(state_vec, cand_tokens, starts, Wq, bq, Wk):
    state_vec = np.ascontiguousarray(np.asarray(state_vec, dtype=np.float32))
    cand_tokens = np.ascontiguousarray(np.asarray(cand_tokens, dtype=np.float32))
    starts = np.asarray(starts)
    Wq = np.ascontiguousarray(np.asarray(Wq, dtype=np.float32))
    bq = np.ascontiguousarray(np.asarray(bq, dtype=np.float32))
    Wk = np.ascontiguousarray(np.asarray(Wk, dtype=np.float32))

    core_ids = list(range(NCORES))
    if _is_uniform(starts):
        nc = _get_prog("fast")
        in_maps = fast_in_maps(state_vec, cand_tokens, Wq, bq, Wk)
        res = run_bass_kernel_spmd(nc, in_maps, core_ids)
        return np.concatenate(
            [np.asarray(res.results[c]["out"]).astype(np.float32)
             for c in core_ids])

    # ---- general path: host derives seg ids / expands kq (index work) ----
    nc = _get_prog("general")
    idx = np.arange(K, dtype=np.int64)
    s64 = starts.astype(np.int64)
    seg = np.searchsorted(s64, idx, side="right") - 1
    seg = np.clip(seg, 0, B - 1)
    valid = (idx >= s64[0]) & (idx < s64[-1])
    kq = ((state_vec @ Wq.T + bq) @ Wk.T).astype(np.float32)
    E = kq[seg]
    E[~valid] = 0.0
    in_maps = []
    for c in range(NCORES):
        in_maps.append({
            "cand": cand_tokens[c * TOK_PER_CORE:(c + 1) * TOK_PER_CORE],
            "E": np.ascontiguousarray(E[c * TOK_PER_CORE:(c + 1) * TOK_PER_CORE]),
        })
    res = run_bass_kernel_spmd(nc, in_maps, core_ids)
    return np.concatenate([res.results[c]["out"] for c in core_ids])

